# revision 26
# baseline (speedup 1.0000x reference)
"""Self-contained Trainium2 Bass kernel for nn_Attention_40226663694923.

Fused LayerNorm + multi-head attention + out-projection, sharded over
8 NeuronCores as (batch b in 0..3) x (head-group g in 0..1, 8 heads each).
Each core receives x[b].T plus its weight shards, computes a partial
out-projection [2048, 1024]; the host sums the two head-group partials
per batch and adds the bias.

v2: restructured for ACT/PE overlap (1016us -> 529us on HW). The softmax
exp (ScalarE ACTIVATE, ~293us/core floor) is the roofline; everything
else pipelines underneath it with no long PE idle gaps (keeps the HAM
clock warm at 2.4GHz):
  - logits land in double-buffered 2-bank PSUM tiles [128,1024]
    (two heads packed via row-group-concurrent matmuls at bases 0/64);
    exp reads PSUM directly, writes bf16 SBUF tiles (bufs=6);
    attnV is emitted with a 2-step lag so the PE queue never blocks
    the logits feed for the next exp, and with deep ex buffering the
    next pair's first exp never waits on this pair's tail attnVs.
  - LN (phase A) and the k/v projections (phase B) are interleaved per
    512-token block; q projections and the out-projection of the
    previous query block are dripped into the attention steps one
    instruction at a time (never in the last 2 steps of a pair).
  - softmax denominators come for free from a ones column in V (row 64
    of the attnV accumulators); -den is broadcast to 128 partitions
    with two K=1 outer-product matmuls (negated selector rows), then
    1/den is computed on the full [128,512] tile with standard DVE ALU
    ops (~bits seed + one Newton step, ~2e-3) — the custom-DVE
    reciprocal_approx_fast op returns garbage on HW in this kernel
    context (works in isolation; root cause not found), so phase C
    avoids it. Normalization is two aligned cross-partition-base DVE
    muls (bases must be 32-aligned; arbitrary bases are rejected).
  - V, attention weights, and the out-projection run in bf16; LN,
    q/k projections and logits stay fp32r (full rate at N>=256).
"""

import os
import sys

for _p in ("/opt/trn_rl_repo", "/root/.axon_site/_ro/trn_rl_repo"):
    if os.path.isdir(_p) and _p not in sys.path:
        sys.path.append(_p)

from collections import deque
from contextlib import ExitStack

import numpy as np

B, N, DIM = 4, 2048, 1024
H, D = 16, 64
HPC = 8        # heads per core
PAIRS = 4      # head pairs per core
KC = 8         # 1024 dim contraction chunks of 128
NB = 4         # token blocks of 512
TB = 512
TT = 16        # token tiles of 128
LN_EPS = 1e-6
N_CORES = 8

_prog_cache = {}


def _build_program():
    import concourse.bass as bass
    import concourse.mybir as mybir
    import concourse.tile as tile
    from concourse import bacc

    F32 = mybir.dt.float32
    F32R = mybir.dt.float32r
    BF16 = mybir.dt.bfloat16
    I32 = mybir.dt.int32
    AF = mybir.ActivationFunctionType
    ALU = mybir.AluOpType

    nc = bacc.Bacc("TRN2", target_bir_lowering=False, debug=False)
    xt_d = nc.dram_tensor("xt", [DIM, N], F32R, kind="ExternalInput")
    wq_d = nc.dram_tensor("wq", [PAIRS, 128, KC, 128], F32R, kind="ExternalInput")
    wk_d = nc.dram_tensor("wk", [128, KC, PAIRS, 128], F32R, kind="ExternalInput")
    wv_d = nc.dram_tensor("wv", [128, KC, 512], F32R, kind="ExternalInput")
    wo_d = nc.dram_tensor("wo", [128, PAIRS, 1024], BF16, kind="ExternalInput")
    qb_d = nc.dram_tensor("qb", [PAIRS, 128], F32, kind="ExternalInput")
    kb_d = nc.dram_tensor("kb", [PAIRS, 128], F32, kind="ExternalInput")
    out_d = nc.dram_tensor("out", [N, DIM], F32, kind="ExternalOutput")

    with tile.TileContext(nc) as tc, ExitStack() as ctx:
        const_p = ctx.enter_context(tc.tile_pool(name="const", bufs=1))
        big_p = ctx.enter_context(tc.tile_pool(name="big", bufs=1))

        onesF = const_p.tile([128, 128], F32)
        nc.vector.memset(onesF, 1.0)
        ones_col = const_p.tile([128, 1], F32R)
        nc.vector.tensor_copy(out=ones_col, in_=onesF[:, 0:1])
        ones_row = const_p.tile([1, 128], F32R)
        nc.vector.tensor_copy(out=ones_row, in_=onesF[0:1, :])
        eps1 = const_p.tile([1, 1], F32)
        nc.vector.memset(eps1, LN_EPS)
        zb128 = const_p.tile([128, 1], F32)
        nc.vector.memset(zb128, 0.0)
        # negated selector rows for the den outer-product broadcasts (the
        # Newton-reciprocal chain below yields +1/den from a -den input)
        self_f = const_p.tile([1, 128], F32)
        nc.vector.memset(self_f, 0.0)
        nc.vector.memset(self_f[0:1, 0:64], -1.0)
        selA = const_p.tile([1, 128], F32R)
        nc.vector.tensor_copy(out=selA, in_=self_f)
        nc.vector.memset(self_f, 0.0)
        nc.vector.memset(self_f[0:1, 64:128], -1.0)
        selB = const_p.tile([1, 128], F32R)
        nc.vector.tensor_copy(out=selB, in_=self_f)
        qb_sb = const_p.tile([128, PAIRS], F32)
        kb_sb = const_p.tile([128, PAIRS], F32)

        # persistent big tensors
        xt_sb = big_p.tile([128, KC, N], F32R)      # becomes z (normalized) in place
        k_sb = big_p.tile([128, PAIRS, N], F32R)    # kT, two heads packed per pair
        v_sb = big_p.tile([128, TT, HPC, D + 1], BF16)  # V natural + ones column
        wo_sb = big_p.tile([128, PAIRS, 1024], BF16)
        wq_sb = big_p.tile([128, PAIRS, KC, 128], F32R)

        for kc in range(6):
            nc.sync.dma_start(out=xt_sb[:, kc, :], in_=xt_d[kc * 128 : (kc + 1) * 128, :])
        for kc in range(6, KC):
            nc.gpsimd.dma_start(out=xt_sb[:, kc, :], in_=xt_d[kc * 128 : (kc + 1) * 128, :])
        nc.sync.dma_start(out=wo_sb, in_=wo_d[:, :, :])
        nc.vector.tensor_copy(
            out=v_sb[:, :, :, D : D + 1],
            in_=onesF.rearrange("p (a b c) -> p a b c", a=TT, b=HPC),
        )
        wkv_cm = tc.tile_pool(name="wkv", bufs=1)
        wkv = wkv_cm.__enter__()
        wv_sb = wkv.tile([128, KC, 512], F32R, tag="wv")
        nc.gpsimd.dma_start(out=wv_sb, in_=wv_d[:, :, :])
        wk_sb = wkv.tile([128, KC, PAIRS, 128], F32R, tag="wk")
        nc.gpsimd.dma_start(out=wk_sb, in_=wk_d[:, :, :, :])
        for pair in range(PAIRS):
            nc.gpsimd.dma_start(
                out=qb_sb[:, pair : pair + 1],
                in_=qb_d[pair, :].rearrange("(p one) -> p one", one=1),
            )
            nc.gpsimd.dma_start(
                out=kb_sb[:, pair : pair + 1],
                in_=kb_d[pair, :].rearrange("(p one) -> p one", one=1),
            )
        for pair in range(PAIRS):
            nc.gpsimd.dma_start(out=wq_sb[:, pair], in_=wq_d[pair])

        # ------- Phases A+B interleaved per token block: LN stats + normalize
        # of xT, then immediately the k/v projections for that block --------
        with tc.tile_pool(name="psA", bufs=2, space="PSUM") as psA, \
             tc.tile_pool(name="sqp", bufs=3) as sqp, \
             tc.tile_pool(name="rows", bufs=2) as rows, \
             tc.tile_pool(name="psB", bufs=2, space="PSUM") as psB:
            def stats(tb):
                ts_ = slice(tb * TB, (tb + 1) * TB)
                s1 = psA.tile([1, TB], F32, tag="s1")
                s2 = psA.tile([1, TB], F32, tag="s2")
                for kc in range(KC):
                    sq = sqp.tile([128, TB], F32R, tag="sq")
                    nc.scalar.activation(out=sq, in_=xt_sb[:, kc, ts_].bitcast(F32),
                                         func=AF.Square, bias=zb128[:, 0:1])
                    nc.tensor.matmul(s1, lhsT=ones_col, rhs=xt_sb[:, kc, ts_],
                                     start=(kc == 0), stop=(kc == KC - 1))
                    nc.tensor.matmul(s2, lhsT=ones_col, rhs=sq,
                                     start=(kc == 0), stop=(kc == KC - 1))
                return s1, s2

            def rows_norm_kv(tb, s1, s2):
                ts_ = slice(tb * TB, (tb + 1) * TB)
                mu = rows.tile([1, TB], F32, tag="mu")
                nc.vector.tensor_scalar_mul(mu, s1, 1.0 / DIM)
                ex2 = rows.tile([1, TB], F32, tag="ex2")
                nc.vector.tensor_scalar_mul(ex2, s2, 1.0 / DIM)
                var_r = rows.tile([1, TB], F32, tag="var")
                nc.vector.tensor_mul(var_r, mu, mu)
                nc.vector.tensor_sub(var_r, ex2, var_r)
                sd = rows.tile([1, TB], F32, tag="sd")
                nc.scalar.activation(out=sd, in_=var_r, func=AF.Sqrt,
                                     bias=eps1[0:1, 0:1])
                rstd = rows.tile([1, TB], F32, tag="rstd")
                nc.vector.reciprocal_approx_fast(out=rstd, in_=sd)
                murstd_r = rows.tile([1, TB], F32R, tag="murstd")
                nc.vector.tensor_mul(murstd_r, mu, rstd)
                rstd_r = rows.tile([1, TB], F32R, tag="rstd_r")
                nc.vector.tensor_copy(out=rstd_r, in_=rstd)
                rb1 = psA.tile([128, TB], F32, tag="rb1", bufs=1)
                nc.tensor.matmul(rb1, lhsT=ones_row, rhs=rstd_r,
                                 start=True, stop=True)
                rb2 = psA.tile([128, TB], F32, tag="rb2", bufs=1)
                nc.tensor.matmul(rb2, lhsT=ones_row, rhs=murstd_r,
                                 start=True, stop=True)
                for kc in range(KC):
                    nc.vector.tensor_mul(xt_sb[:, kc, ts_],
                                         xt_sb[:, kc, ts_].bitcast(F32), rb1)
                    nc.vector.tensor_sub(xt_sb[:, kc, ts_],
                                         xt_sb[:, kc, ts_].bitcast(F32), rb2)
                for pair in range(PAIRS):
                    pk = psB.tile([128, TB], F32, tag="acc")
                    for kc in range(KC):
                        nc.tensor.matmul(pk, lhsT=wk_sb[:, kc, pair, :],
                                         rhs=xt_sb[:, kc, ts_],
                                         start=(kc == 0), stop=(kc == KC - 1))
                    nc.vector.tensor_scalar_add(out=k_sb[:, pair, ts_], in0=pk,
                                                scalar1=kb_sb[:, pair : pair + 1])
                for tt in range(tb * 4, tb * 4 + 4):
                    tts = slice(tt * 128, (tt + 1) * 128)
                    pv = psB.tile([128, 512], F32, tag="acc")
                    for kc in range(KC):
                        nc.tensor.matmul(pv, lhsT=xt_sb[:, kc, tts],
                                         rhs=wv_sb[:, kc, :],
                                         start=(kc == 0), stop=(kc == KC - 1))
                    nc.vector.tensor_copy(
                        out=v_sb[:, tt, :, 0:D],
                        in_=pv.rearrange("p (h d) -> p h d", h=HPC),
                    )

            # stats staggered one block ahead: its squares keep the ACT FIFO
            # from stalling behind the previous block's Sqrt
            ss = {0: stats(0)}
            for tb in range(NB):
                if tb + 1 < NB:
                    ss[tb + 1] = stats(tb + 1)
                rows_norm_kv(tb, *ss.pop(tb))
        wkv_cm.__exit__(None, None, None)

        # ---------------- Phase C: pipelined attention + out-projection ------
        psC = ctx.enter_context(tc.tile_pool(name="psC", bufs=1, space="PSUM"))
        exp_p = ctx.enter_context(tc.tile_pool(name="exp", bufs=8))
        qp_ = ctx.enter_context(tc.tile_pool(name="qp", bufs=2))
        ob_p = ctx.enter_context(tc.tile_pool(name="ob", bufs=2))
        osb_p = ctx.enter_context(tc.tile_pool(name="osb", bufs=2))
        rec_p = ctx.enter_context(tc.tile_pool(name="rec", bufs=2))

        drip = deque()

        def drip_pop():
            n = 2 if len(drip) > 10 else 1
            for _ in range(n):
                if drip:
                    drip.popleft()()

        def make_qproj_ops(tqb, pair, sink):
            """Closures: 8 accumulating matmuls + bias add -> q tile."""
            cell = {}
            ops = []
            tqs_ = slice(tqb * TB, (tqb + 1) * TB)

            def mk(kc):
                def op():
                    if kc == 0:
                        cell["pq"] = psC.tile([128, TB], F32, tag="pq", bufs=1, name="pq")
                    nc.tensor.matmul(cell["pq"], lhsT=wq_sb[:, pair, kc, :],
                                     rhs=xt_sb[:, kc, tqs_],
                                     start=(kc == 0), stop=(kc == KC - 1))
                return op

            for kc in range(KC):
                ops.append(mk(kc))

            def bias_op():
                q_t = qp_.tile([128, TB], F32R, tag="q")
                nc.vector.tensor_scalar_add(out=q_t, in0=cell["pq"],
                                            scalar1=qb_sb[:, pair : pair + 1])
                sink[(tqb, pair)] = q_t
            ops.append(bias_op)
            return ops

        RC0, RC1, RC2 = -0.23549792, 2.0017324, 2.0

        def make_pairend_ops(pair, po0, po1, obuf):
            """den rows -> broadcast -den -> Newton 1/den -> scaled O."""
            def op1():
                recr = rec_p.tile([1, 1024], F32R, tag="recr")
                nc.vector.tensor_copy(out=recr[0:1, 0:512],
                                      in_=po0[D : D + 1, :])
                nc.vector.tensor_copy(out=recr[0:1, 512:1024],
                                      in_=po1[D : D + 1, :])
                # dnb rows 0-63 = -den0, rows 64-127 = -den1
                dnb = psC.tile([128, 512], F32, tag="misc", bufs=1)
                nc.tensor.matmul(dnb, lhsT=selA, rhs=recr[0:1, 0:512],
                                 start=True, stop=False)
                nc.tensor.matmul(dnb, lhsT=selB, rhs=recr[0:1, 512:1024],
                                 start=False, stop=True)
                # 1/den = chain(-den): ~bits seed + 1 Newton step (~2e-3)
                w1 = rec_p.tile([128, 512], F32, tag="w1", bufs=1)
                nc.vector.tensor_scalar(out=w1.bitcast(I32),
                                        in0=dnb.bitcast(I32), scalar1=-1,
                                        scalar2=None, op0=ALU.bitwise_xor)
                nc.vector.tensor_scalar_mul(w1, w1, RC0)
                w2 = rec_p.tile([128, 512], F32, tag="w2", bufs=1)
                nc.vector.tensor_mul(w2, dnb, w1)
                rb_sb = rec_p.tile([128, 512], F32, tag="rbs")
                nc.vector.scalar_tensor_tensor(out=rb_sb, in0=w2, scalar=RC1,
                                               in1=w1, op0=ALU.subtract,
                                               op1=ALU.mult)
                nc.vector.tensor_mul(obuf[0:64, pair, :], po0[0:D, :],
                                     rb_sb[0:64, :])
                nc.vector.tensor_mul(obuf[64:128, pair, :], po1[0:D, :],
                                     rb_sb[64:128, :])
            return [op1]

        def make_outproj_ops(tqb, obuf, tags=("misc",)):
            """8 groups of (4 matmuls + copy); DMA per 128-token row block."""
            ops = []
            for tqs in range(4):
                cell = {}

                def mk_alloc(cell=cell):
                    def op():
                        cell["osb"] = osb_p.tile([128, 1024], F32, tag="osb", name="osb")
                    return op
                ops.append(mk_alloc())
                for nh in range(2):
                    for j in range(PAIRS):
                        def mk_mmj(tqs=tqs, nh=nh, j=j, cell=cell):
                            def op():
                                if j == 0:
                                    cell["pc"] = psC.tile([128, 512], F32, tag="misc",
                                                          bufs=1, name="pc")
                                nc.tensor.matmul(
                                    cell["pc"],
                                    lhsT=obuf[:, j, tqs * 128 : (tqs + 1) * 128],
                                    rhs=wo_sb[:, j, nh * 512 : (nh + 1) * 512],
                                    start=(j == 0), stop=(j == PAIRS - 1))
                            return op
                        ops.append(mk_mmj())

                    def mk_copy(tqs=tqs, nh=nh, cell=cell):
                        def op():
                            nc.vector.tensor_copy(
                                out=cell["osb"][:, nh * 512 : (nh + 1) * 512],
                                in_=cell["pc"])
                            if nh == 1:
                                r0 = tqb * TB + tqs * 128
                                nc.sync.dma_start(out=out_d[r0 : r0 + 128, :],
                                                  in_=cell["osb"])
                        return op
                    ops.append(mk_copy())
            return ops

        q_tiles = {}
        pairseq = [(t, p) for t in range(NB) for p in range(PAIRS)]
        # q projection for the very first pair runs inline
        for op in make_qproj_ops(0, 0, q_tiles):
            op()

        # one flat software-pipelined stream over all (tqb, pair) x tkc steps:
        # logits/exp of the next pair are emitted before the tail attnVs of
        # the current pair, so the exp feed never sees a pair boundary
        ctxs = {}
        obuf = None
        prev_obuf = None
        NSTEP = len(pairseq) * TT + 2
        for g in range(NSTEP):
            i, tkc = divmod(g, TT)
            if i < len(pairseq) and tkc == 0:
                tqb, pair = pairseq[i]
                if pair == 0:
                    obuf = ob_p.tile([128, PAIRS, 512], BF16, tag="ob",
                                     name="obuf")
                    if tqb > 0:
                        drip.extend(make_outproj_ops(tqb - 1, prev_obuf))
                if i + 1 < len(pairseq):
                    nt, np_ = pairseq[i + 1]
                    drip.extendleft(reversed(make_qproj_ops(nt, np_, q_tiles)))
                while pairseq[i] not in q_tiles:
                    drip.popleft()()
                po0 = psC.tile([128, 512], F32, tag="po", bufs=2, name="po0")
                po1 = psC.tile([128, 512], F32, tag="po", bufs=2, name="po1")
                ctxs[i] = {"pair": pair, "q_t": q_tiles.pop(pairseq[i]),
                           "po0": po0, "po1": po1, "obuf": obuf, "ex": {}}
                if pair == PAIRS - 1:
                    prev_obuf = obuf
            if i < len(pairseq):
                c = ctxs[i]
                pl = psC.tile([128, 1024], F32, tag="pl", bufs=2)
                nc.tensor.matmul(
                    pl[:, 0:512],
                    lhsT=k_sb[0:64, c["pair"], tkc * 128 : (tkc + 1) * 128],
                    rhs=c["q_t"][0:64, :], start=True, stop=True)
                nc.tensor.matmul(
                    pl[:, 512:1024],
                    lhsT=k_sb[64:128, c["pair"], tkc * 128 : (tkc + 1) * 128],
                    rhs=c["q_t"][64:128, :], start=True, stop=True)
                ex = exp_p.tile([128, 1024], BF16, tag="ex")
                nc.scalar.activation(out=ex, in_=pl, func=AF.Exp,
                                     bias=zb128[:, 0:1])
                c["ex"][tkc] = ex
            if g >= 2:
                i2, tk2 = divmod(g - 2, TT)
                c2 = ctxs[i2]
                ex = c2["ex"].pop(tk2)
                nc.tensor.matmul(c2["po0"][0 : D + 1, :],
                                 lhsT=v_sb[:, tk2, c2["pair"] * 2, :],
                                 rhs=ex[:, 0:512],
                                 start=(tk2 == 0), stop=(tk2 == TT - 1))
                nc.tensor.matmul(c2["po1"][0 : D + 1, :],
                                 lhsT=v_sb[:, tk2, c2["pair"] * 2 + 1, :],
                                 rhs=ex[:, 512:1024],
                                 start=(tk2 == 0), stop=(tk2 == TT - 1))
                if tk2 == TT - 1:
                    drip.extendleft(reversed(make_pairend_ops(
                        c2["pair"], c2["po0"], c2["po1"], c2["obuf"])))
                    del ctxs[i2]
            drip_pop()

        # drain: leftover drip (includes the last pair's normalization),
        # then the final out-projection
        while drip:
            drip.popleft()()
        for op in make_outproj_ops(NB - 1, prev_obuf, tags=("misc", "po")):
            op()

    nc.finalize()
    return nc


def get_program():
    if "nc" not in _prog_cache:
        _prog_cache["nc"] = _build_program()
    return _prog_cache["nc"]


def _round_f32r(a):
    """Round fp32 to fp32r (E8M11: 11 mantissa bits, low 12 bits zero),
    round-to-nearest-even. Matches the PE's fp32r operand precision so the
    DMA-loaded tensors satisfy walrus's 'rounded to FP32r' requirement."""
    b = np.ascontiguousarray(a, np.float32).view(np.uint32)
    lsb = (b >> np.uint32(12)) & np.uint32(1)
    r = (b + np.uint32(0x7FF) + lsb) & np.uint32(0xFFFFF000)
    return r.view(np.float32)


def _pack_inputs(x, ln_scale, ln_bias, w_qkv, w_out, b_out):
    """Returns (in_maps for 8 cores, per-batch host bias [1024])."""
    import ml_dtypes

    x = np.ascontiguousarray(np.asarray(x, np.float32))
    ln_scale = np.asarray(ln_scale, np.float32)
    ln_bias = np.asarray(ln_bias, np.float32)
    w_qkv = np.asarray(w_qkv, np.float32)
    w_out = np.asarray(w_out, np.float32)
    b_out = np.asarray(b_out, np.float32)

    ws = w_qkv * ln_scale[:, None]          # fold LN scale into weights
    wq_all = ws[:, 0:1024] * (D ** -0.5)    # fold 1/sqrt(d) into q
    wk_all = ws[:, 1024:2048]
    wv_all = ws[:, 2048:3072]
    qb_all = (ln_bias @ w_qkv[:, 0:1024]) * (D ** -0.5)
    kb_all = ln_bias @ w_qkv[:, 1024:2048]
    vb_all = ln_bias @ w_qkv[:, 2048:3072]
    b_eff = (b_out + vb_all @ w_out).astype(np.float32)  # host-side bias

    in_maps = []
    for core in range(N_CORES):
        b_i, g = core // 2, core % 2
        cs = slice(g * 512, (g + 1) * 512)
        # [dim, 8 heads, 64] -> pairs of heads packed along m
        wq_g = wq_all[:, cs].reshape(DIM, PAIRS, 128)   # [dim, pair, 2*64]
        wk_g = wk_all[:, cs].reshape(DIM, PAIRS, 128)
        # -> [pair, p, kc, m] so that per-pair DMA is contiguous per partition
        wq_p = np.ascontiguousarray(
            wq_g.reshape(KC, 128, PAIRS, 128).transpose(2, 1, 0, 3))
        wk_p = np.ascontiguousarray(
            wk_g.reshape(KC, 128, PAIRS, 128).transpose(1, 0, 2, 3))
        wv_p = np.ascontiguousarray(
            wv_all[:, cs].reshape(KC, 128, 512).transpose(1, 0, 2))
        wo_p = np.ascontiguousarray(
            w_out[cs, :].reshape(PAIRS, 128, DIM).transpose(1, 0, 2))
        qb_p = np.ascontiguousarray(qb_all[cs].reshape(PAIRS, 128))
        kb_p = np.ascontiguousarray(kb_all[cs].reshape(PAIRS, 128))
        xt = np.ascontiguousarray(x[b_i].T)
        in_maps.append({
            "xt": _round_f32r(xt), "wq": _round_f32r(wq_p),
            "wk": _round_f32r(wk_p), "wv": _round_f32r(wv_p),
            "wo": wo_p.astype(ml_dtypes.bfloat16), "qb": qb_p, "kb": kb_p,
        })
    return in_maps, b_eff


def kernel(x, ln_scale, ln_bias, w_qkv, w_out, b_out):
    from concourse.bass_utils import run_bass_kernel_spmd

    nc = get_program()
    in_maps, b_eff = _pack_inputs(x, ln_scale, ln_bias, w_qkv, w_out, b_out)
    trace = bool(os.environ.get("ATTN_KERNEL_TRACE"))
    res = run_bass_kernel_spmd(nc, in_maps, core_ids=list(range(N_CORES)),
                               trace=trace)
    _prog_cache["last_exec_time_ns"] = res.exec_time_ns
    outs = res.results
    out = np.empty((B, N, DIM), np.float32)
    for b in range(B):
        out[b] = outs[2 * b]["out"] + outs[2 * b + 1]["out"] + b_eff
    return out


# revision 29
# speedup vs baseline: 1.1924x; 1.1924x over previous
"""Self-contained Trainium2 Bass kernel for nn_Attention_40226663694923.

Fused LayerNorm + multi-head attention + out-projection, sharded over
8 NeuronCores as (batch b in 0..3) x (head-group g in 0..1, 8 heads each).
Each core receives x[b].T plus its weight shards, computes a partial
out-projection [2048, 1024]; the host sums the two head-group partials
per batch and adds the bias.

v2: restructured for ACT/PE overlap (1016us -> 529us on HW). The softmax
exp (ScalarE ACTIVATE, ~293us/core floor) is the roofline; everything
else pipelines underneath it with no long PE idle gaps (keeps the HAM
clock warm at 2.4GHz):
  - logits land in double-buffered 2-bank PSUM tiles [128,1024]
    (two heads packed via row-group-concurrent matmuls at bases 0/64);
    exp reads PSUM directly, writes bf16 SBUF tiles (bufs=6);
    attnV is emitted with a 2-step lag so the PE queue never blocks
    the logits feed for the next exp, and with deep ex buffering the
    next pair's first exp never waits on this pair's tail attnVs.
  - LN (phase A) and the k/v projections (phase B) are interleaved per
    512-token block; q projections and the out-projection of the
    previous query block are dripped into the attention steps one
    instruction at a time (never in the last 2 steps of a pair).
  - softmax denominators come for free from a ones column in V (row 64
    of the attnV accumulators); -den is broadcast to 128 partitions
    with two K=1 outer-product matmuls (negated selector rows), then
    1/den is computed on the full [128,512] tile with standard DVE ALU
    ops (~bits seed + one Newton step, ~2e-3) — the custom-DVE
    reciprocal_approx_fast op returns garbage on HW in this kernel
    context (works in isolation; root cause not found), so phase C
    avoids it. Normalization is two aligned cross-partition-base DVE
    muls (bases must be 32-aligned; arbitrary bases are rejected).
  - V, attention weights, and the out-projection run in bf16; LN,
    q/k projections and logits stay fp32r (full rate at N>=256).
"""

import os
import sys

for _p in ("/opt/trn_rl_repo", "/root/.axon_site/_ro/trn_rl_repo"):
    if os.path.isdir(_p) and _p not in sys.path:
        sys.path.append(_p)

from collections import deque
from contextlib import ExitStack

import numpy as np

B, N, DIM = 4, 2048, 1024
H, D = 16, 64
HPC = 8        # heads per core
PAIRS = 4      # head pairs per core
KC = 8         # 1024 dim contraction chunks of 128
NB = 4         # token blocks of 512
TB = 512
TT = 16        # token tiles of 128
LN_EPS = 1e-6
N_CORES = 8

_prog_cache = {}


def _build_program():
    import concourse.bass as bass
    import concourse.mybir as mybir
    import concourse.tile as tile
    from concourse import bacc

    F32 = mybir.dt.float32
    F32R = mybir.dt.float32r
    BF16 = mybir.dt.bfloat16
    I32 = mybir.dt.int32
    AF = mybir.ActivationFunctionType
    ALU = mybir.AluOpType

    nc = bacc.Bacc("TRN2", target_bir_lowering=False, debug=False)
    xt_d = nc.dram_tensor("xt", [DIM, N], F32R, kind="ExternalInput")
    wq_d = nc.dram_tensor("wq", [PAIRS, 128, KC, 128], F32R, kind="ExternalInput")
    wk_d = nc.dram_tensor("wk", [128, KC, PAIRS, 128], F32R, kind="ExternalInput")
    wv_d = nc.dram_tensor("wv", [128, KC, 512], F32R, kind="ExternalInput")
    wo_d = nc.dram_tensor("wo", [128, PAIRS, 1024], BF16, kind="ExternalInput")
    qb_d = nc.dram_tensor("qb", [PAIRS, 128], F32, kind="ExternalInput")
    kb_d = nc.dram_tensor("kb", [PAIRS, 128], F32, kind="ExternalInput")
    out_d = nc.dram_tensor("out", [N, DIM], F32, kind="ExternalOutput")

    with tile.TileContext(nc) as tc, ExitStack() as ctx:
        const_p = ctx.enter_context(tc.tile_pool(name="const", bufs=1))
        big_p = ctx.enter_context(tc.tile_pool(name="big", bufs=1))

        onesF = const_p.tile([128, 128], F32)
        nc.vector.memset(onesF, 1.0)
        ones_col = const_p.tile([128, 1], F32R)
        nc.vector.tensor_copy(out=ones_col, in_=onesF[:, 0:1])
        ones_row = const_p.tile([1, 128], F32R)
        nc.vector.tensor_copy(out=ones_row, in_=onesF[0:1, :])
        eps1 = const_p.tile([1, 1], F32)
        nc.vector.memset(eps1, LN_EPS)
        zb128 = const_p.tile([128, 1], F32)
        nc.vector.memset(zb128, 0.0)
        # negated selector rows for the den outer-product broadcasts (the
        # Newton-reciprocal chain below yields +1/den from a -den input)
        self_f = const_p.tile([1, 128], F32)
        nc.vector.memset(self_f, 0.0)
        nc.vector.memset(self_f[0:1, 0:64], -1.0)
        selA = const_p.tile([1, 128], F32R)
        nc.vector.tensor_copy(out=selA, in_=self_f)
        nc.vector.memset(self_f, 0.0)
        nc.vector.memset(self_f[0:1, 64:128], -1.0)
        selB = const_p.tile([1, 128], F32R)
        nc.vector.tensor_copy(out=selB, in_=self_f)
        qb_sb = const_p.tile([128, PAIRS], F32)
        kb_sb = const_p.tile([128, PAIRS], F32)
        q00_sb = const_p.tile([128, TB], F32R)

        # persistent big tensors
        xt_sb = big_p.tile([128, KC, N], F32R)      # becomes z (normalized) in place
        k_sb = big_p.tile([128, PAIRS, N], F32R)    # kT, two heads packed per pair
        v_sb = big_p.tile([128, TT, HPC, D + 1], BF16)  # V natural + ones column
        wo_sb = big_p.tile([128, PAIRS, 1024], BF16)
        wq_sb = big_p.tile([128, PAIRS, KC, 128], F32R)

        for kc in range(6):
            nc.sync.dma_start(out=xt_sb[:, kc, :], in_=xt_d[kc * 128 : (kc + 1) * 128, :])
        for kc in range(6, KC):
            nc.gpsimd.dma_start(out=xt_sb[:, kc, :], in_=xt_d[kc * 128 : (kc + 1) * 128, :])
        nc.sync.dma_start(out=wo_sb, in_=wo_d[:, :, :])
        nc.vector.tensor_copy(
            out=v_sb[:, :, :, D : D + 1],
            in_=onesF.rearrange("p (a b c) -> p a b c", a=TT, b=HPC),
        )
        wkv_cm = tc.tile_pool(name="wkv", bufs=1)
        wkv = wkv_cm.__enter__()
        wv_sb = wkv.tile([128, KC, 512], F32R, tag="wv")
        nc.gpsimd.dma_start(out=wv_sb, in_=wv_d[:, :, :])
        wk_sb = wkv.tile([128, KC, PAIRS, 128], F32R, tag="wk")
        nc.gpsimd.dma_start(out=wk_sb, in_=wk_d[:, :, :, :])
        for pair in range(PAIRS):
            nc.gpsimd.dma_start(
                out=qb_sb[:, pair : pair + 1],
                in_=qb_d[pair, :].rearrange("(p one) -> p one", one=1),
            )
            nc.gpsimd.dma_start(
                out=kb_sb[:, pair : pair + 1],
                in_=kb_d[pair, :].rearrange("(p one) -> p one", one=1),
            )
        for pair in range(PAIRS):
            nc.gpsimd.dma_start(out=wq_sb[:, pair], in_=wq_d[pair])

        q_tiles = {}

        # ------- Phases A+B interleaved per token block: LN stats + normalize
        # of xT, then immediately the k/v projections for that block --------
        with tc.tile_pool(name="psA", bufs=2, space="PSUM") as psA, \
             tc.tile_pool(name="sqp", bufs=3) as sqp, \
             tc.tile_pool(name="rows", bufs=2) as rows, \
             tc.tile_pool(name="psB", bufs=2, space="PSUM") as psB:
            def stats(tb):
                ts_ = slice(tb * TB, (tb + 1) * TB)
                s1 = psA.tile([1, TB], F32, tag="s1")
                s2 = psA.tile([1, TB], F32, tag="s2")
                for kc in range(KC):
                    sq = sqp.tile([128, TB], F32R, tag="sq")
                    nc.scalar.activation(out=sq, in_=xt_sb[:, kc, ts_].bitcast(F32),
                                         func=AF.Square, bias=zb128[:, 0:1])
                    nc.tensor.matmul(s1, lhsT=ones_col, rhs=xt_sb[:, kc, ts_],
                                     start=(kc == 0), stop=(kc == KC - 1))
                    nc.tensor.matmul(s2, lhsT=ones_col, rhs=sq,
                                     start=(kc == 0), stop=(kc == KC - 1))
                return s1, s2

            def rows_norm_kv(tb, s1, s2):
                ts_ = slice(tb * TB, (tb + 1) * TB)
                mu = rows.tile([1, TB], F32, tag="mu")
                nc.vector.tensor_scalar_mul(mu, s1, 1.0 / DIM)
                ex2 = rows.tile([1, TB], F32, tag="ex2")
                nc.vector.tensor_scalar_mul(ex2, s2, 1.0 / DIM)
                var_r = rows.tile([1, TB], F32, tag="var")
                nc.vector.tensor_mul(var_r, mu, mu)
                nc.vector.tensor_sub(var_r, ex2, var_r)
                sd = rows.tile([1, TB], F32, tag="sd")
                nc.scalar.activation(out=sd, in_=var_r, func=AF.Sqrt,
                                     bias=eps1[0:1, 0:1])
                rstd = rows.tile([1, TB], F32, tag="rstd")
                nc.vector.reciprocal_approx_fast(out=rstd, in_=sd)
                murstd_r = rows.tile([1, TB], F32R, tag="murstd")
                nc.vector.tensor_mul(murstd_r, mu, rstd)
                rstd_r = rows.tile([1, TB], F32R, tag="rstd_r")
                nc.vector.tensor_copy(out=rstd_r, in_=rstd)
                rb1 = psA.tile([128, TB], F32, tag="rb1", bufs=1)
                nc.tensor.matmul(rb1, lhsT=ones_row, rhs=rstd_r,
                                 start=True, stop=True)
                rb2 = psA.tile([128, TB], F32, tag="rb2", bufs=1)
                nc.tensor.matmul(rb2, lhsT=ones_row, rhs=murstd_r,
                                 start=True, stop=True)
                for kc in range(KC):
                    nc.vector.tensor_mul(xt_sb[:, kc, ts_],
                                         xt_sb[:, kc, ts_].bitcast(F32), rb1)
                    nc.vector.tensor_sub(xt_sb[:, kc, ts_],
                                         xt_sb[:, kc, ts_].bitcast(F32), rb2)
                for pair in range(PAIRS):
                    pk = psB.tile([128, TB], F32, tag="acc")
                    for kc in range(KC):
                        nc.tensor.matmul(pk, lhsT=wk_sb[:, kc, pair, :],
                                         rhs=xt_sb[:, kc, ts_],
                                         start=(kc == 0), stop=(kc == KC - 1))
                    nc.vector.tensor_scalar_add(out=k_sb[:, pair, ts_], in0=pk,
                                                scalar1=kb_sb[:, pair : pair + 1])
                for tt in range(tb * 4, tb * 4 + 4):
                    tts = slice(tt * 128, (tt + 1) * 128)
                    pv = psB.tile([128, 512], F32, tag="acc")
                    for kc in range(KC):
                        nc.tensor.matmul(pv, lhsT=xt_sb[:, kc, tts],
                                         rhs=wv_sb[:, kc, :],
                                         start=(kc == 0), stop=(kc == KC - 1))
                    nc.vector.tensor_copy(
                        out=v_sb[:, tt, :, 0:D],
                        in_=pv.rearrange("p (h d) -> p h d", h=HPC),
                    )

            # stats staggered one block ahead: its squares keep the ACT FIFO
            # from stalling behind the previous block's Sqrt
            ss = {0: stats(0)}
            for tb in range(NB):
                if tb + 1 < NB:
                    ss[tb + 1] = stats(tb + 1)
                rows_norm_kv(tb, *ss.pop(tb))
                if tb == 0:
                    # q projection for the very first pair, overlapped with B
                    pq0 = psB.tile([128, TB], F32, tag="acc")
                    for kc in range(KC):
                        nc.tensor.matmul(pq0, lhsT=wq_sb[:, 0, kc, :],
                                         rhs=xt_sb[:, kc, 0:TB],
                                         start=(kc == 0), stop=(kc == KC - 1))
                    nc.vector.tensor_scalar_add(out=q00_sb, in0=pq0,
                                                scalar1=qb_sb[:, 0:1])
                    q_tiles[(0, 0)] = q00_sb
            # prewarm the Exp table set while the ACT engine is idle in B
            exwarm = sqp.tile([128, TB], F32R, tag="sq")
            nc.scalar.activation(out=exwarm[:, 0:1], in_=zb128[:, 0:1],
                                 func=AF.Exp, bias=zb128[:, 0:1])
        wkv_cm.__exit__(None, None, None)

        # ---------------- Phase C: pipelined attention + out-projection ------
        psC = ctx.enter_context(tc.tile_pool(name="psC", bufs=1, space="PSUM"))
        exp_p = ctx.enter_context(tc.tile_pool(name="exp", bufs=6))
        qp_ = ctx.enter_context(tc.tile_pool(name="qp", bufs=2))
        ob_p = ctx.enter_context(tc.tile_pool(name="ob", bufs=2))
        osb_p = ctx.enter_context(tc.tile_pool(name="osb", bufs=2))
        rec_p = ctx.enter_context(tc.tile_pool(name="rec", bufs=2))

        drip = deque()

        def drip_pop():
            n = 2 if len(drip) > 10 else 1
            for _ in range(n):
                if drip:
                    drip.popleft()()

        def make_qproj_ops(tqb, pair, sink):
            """Closures: 8 accumulating matmuls + bias add -> q tile."""
            cell = {}
            ops = []
            tqs_ = slice(tqb * TB, (tqb + 1) * TB)

            def mk(kc):
                def op():
                    if kc == 0:
                        cell["pq"] = psC.tile([128, TB], F32, tag="pq", bufs=1, name="pq")
                    nc.tensor.matmul(cell["pq"], lhsT=wq_sb[:, pair, kc, :],
                                     rhs=xt_sb[:, kc, tqs_],
                                     start=(kc == 0), stop=(kc == KC - 1))
                return op

            for kc in range(KC):
                ops.append(mk(kc))

            def bias_op():
                q_t = qp_.tile([128, TB], F32R, tag="q")
                nc.vector.tensor_scalar_add(out=q_t, in0=cell["pq"],
                                            scalar1=qb_sb[:, pair : pair + 1])
                sink[(tqb, pair)] = q_t
            ops.append(bias_op)
            return ops

        RC0, RC1, RC2 = -0.23549792, 2.0017324, 2.0

        def make_pairend_ops(pair, po0, po1, obuf):
            """den rows -> broadcast -den -> Newton 1/den -> scaled O."""
            def op1():
                recr = rec_p.tile([1, 1024], F32R, tag="recr")
                nc.vector.tensor_copy(out=recr[0:1, 0:512],
                                      in_=po0[D : D + 1, :])
                nc.vector.tensor_copy(out=recr[0:1, 512:1024],
                                      in_=po1[D : D + 1, :])
                # dnb rows 0-63 = -den0, rows 64-127 = -den1
                dnb = psC.tile([128, 512], F32, tag="misc", bufs=1)
                nc.tensor.matmul(dnb, lhsT=selA, rhs=recr[0:1, 0:512],
                                 start=True, stop=False)
                nc.tensor.matmul(dnb, lhsT=selB, rhs=recr[0:1, 512:1024],
                                 start=False, stop=True)
                # 1/den = chain(-den): ~bits seed + 1 Newton step (~2e-3)
                w1 = rec_p.tile([128, 512], F32, tag="w1", bufs=1)
                nc.vector.tensor_scalar(out=w1.bitcast(I32),
                                        in0=dnb.bitcast(I32), scalar1=-1,
                                        scalar2=None, op0=ALU.bitwise_xor)
                nc.vector.tensor_scalar_mul(w1, w1, RC0)
                w2 = rec_p.tile([128, 512], F32, tag="w2", bufs=1)
                nc.vector.tensor_mul(w2, dnb, w1)
                rb_sb = rec_p.tile([128, 512], F32, tag="rbs")
                nc.vector.scalar_tensor_tensor(out=rb_sb, in0=w2, scalar=RC1,
                                               in1=w1, op0=ALU.subtract,
                                               op1=ALU.mult)
                nc.vector.tensor_mul(obuf[0:64, pair, :], po0[0:D, :],
                                     rb_sb[0:64, :])
                nc.vector.tensor_mul(obuf[64:128, pair, :], po1[0:D, :],
                                     rb_sb[64:128, :])
            return [op1]

        def make_outproj_ops(tqb, obuf, tags=("misc",)):
            """8 groups of (4 matmuls + copy); DMA per 128-token row block."""
            ops = []
            for tqs in range(4):
                cell = {}

                def mk_alloc(cell=cell):
                    def op():
                        cell["osb"] = osb_p.tile([128, 1024], F32, tag="osb", name="osb")
                    return op
                ops.append(mk_alloc())
                for nh in range(2):
                    for j in range(PAIRS):
                        def mk_mmj(tqs=tqs, nh=nh, j=j, cell=cell):
                            def op():
                                if j == 0:
                                    cell["pc"] = psC.tile([128, 512], F32, tag="misc",
                                                          bufs=1, name="pc")
                                nc.tensor.matmul(
                                    cell["pc"],
                                    lhsT=obuf[:, j, tqs * 128 : (tqs + 1) * 128],
                                    rhs=wo_sb[:, j, nh * 512 : (nh + 1) * 512],
                                    start=(j == 0), stop=(j == PAIRS - 1))
                            return op
                        ops.append(mk_mmj())

                    def mk_copy(tqs=tqs, nh=nh, cell=cell):
                        def op():
                            nc.vector.tensor_copy(
                                out=cell["osb"][:, nh * 512 : (nh + 1) * 512],
                                in_=cell["pc"])
                            if nh == 1:
                                r0 = tqb * TB + tqs * 128
                                nc.sync.dma_start(out=out_d[r0 : r0 + 128, :],
                                                  in_=cell["osb"])
                        return op
                    ops.append(mk_copy())
            return ops

        pairseq = [(t, p) for t in range(NB) for p in range(PAIRS)]

        # one flat software-pipelined stream over all (tqb, pair) x tkc steps:
        # logits/exp of the next pair are emitted before the tail attnVs of
        # the current pair, so the exp feed never sees a pair boundary
        ctxs = {}
        obuf = None
        prev_obuf = None
        NSTEP = len(pairseq) * TT + 2
        for g in range(NSTEP):
            i, tkc = divmod(g, TT)
            if i < len(pairseq) and tkc == 0:
                tqb, pair = pairseq[i]
                if pair == 0:
                    obuf = ob_p.tile([128, PAIRS, 512], BF16, tag="ob",
                                     name="obuf")
                    if tqb > 0:
                        drip.extend(make_outproj_ops(tqb - 1, prev_obuf))
                if i + 1 < len(pairseq):
                    nt, np_ = pairseq[i + 1]
                    drip.extendleft(reversed(make_qproj_ops(nt, np_, q_tiles)))
                while pairseq[i] not in q_tiles:
                    drip.popleft()()
                po0 = psC.tile([128, 512], F32, tag="po", bufs=2, name="po0")
                po1 = psC.tile([128, 512], F32, tag="po", bufs=2, name="po1")
                ctxs[i] = {"pair": pair, "q_t": q_tiles.pop(pairseq[i]),
                           "po0": po0, "po1": po1, "obuf": obuf, "ex": {}}
                if pair == PAIRS - 1:
                    prev_obuf = obuf
            if i < len(pairseq):
                c = ctxs[i]
                pl = psC.tile([128, 1024], F32, tag="pl", bufs=2)
                nc.tensor.matmul(
                    pl[:, 0:512],
                    lhsT=k_sb[0:64, c["pair"], tkc * 128 : (tkc + 1) * 128],
                    rhs=c["q_t"][0:64, :], start=True, stop=True)
                nc.tensor.matmul(
                    pl[:, 512:1024],
                    lhsT=k_sb[64:128, c["pair"], tkc * 128 : (tkc + 1) * 128],
                    rhs=c["q_t"][64:128, :], start=True, stop=True)
                ex = exp_p.tile([128, 1024], BF16, tag="ex")
                nc.scalar.activation(out=ex, in_=pl, func=AF.Exp,
                                     bias=zb128[:, 0:1])
                c["ex"][tkc] = ex
            if g >= 2:
                i2, tk2 = divmod(g - 2, TT)
                c2 = ctxs[i2]
                ex = c2["ex"].pop(tk2)
                nc.tensor.matmul(c2["po0"][0 : D + 1, :],
                                 lhsT=v_sb[:, tk2, c2["pair"] * 2, :],
                                 rhs=ex[:, 0:512],
                                 start=(tk2 == 0), stop=(tk2 == TT - 1))
                nc.tensor.matmul(c2["po1"][0 : D + 1, :],
                                 lhsT=v_sb[:, tk2, c2["pair"] * 2 + 1, :],
                                 rhs=ex[:, 512:1024],
                                 start=(tk2 == 0), stop=(tk2 == TT - 1))
                if tk2 == TT - 1:
                    drip.extendleft(reversed(make_pairend_ops(
                        c2["pair"], c2["po0"], c2["po1"], c2["obuf"])))
                    del ctxs[i2]
            drip_pop()

        # drain: leftover drip (includes the last pair's normalization),
        # then the final out-projection
        while drip:
            drip.popleft()()
        for op in make_outproj_ops(NB - 1, prev_obuf, tags=("misc", "po")):
            op()

    nc.finalize()
    return nc


def get_program():
    if "nc" not in _prog_cache:
        _prog_cache["nc"] = _build_program()
    return _prog_cache["nc"]


def _round_f32r(a):
    """Round fp32 to fp32r (E8M11: 11 mantissa bits, low 12 bits zero),
    round-to-nearest-even. Matches the PE's fp32r operand precision so the
    DMA-loaded tensors satisfy walrus's 'rounded to FP32r' requirement."""
    b = np.ascontiguousarray(a, np.float32).view(np.uint32)
    lsb = (b >> np.uint32(12)) & np.uint32(1)
    r = (b + np.uint32(0x7FF) + lsb) & np.uint32(0xFFFFF000)
    return r.view(np.float32)


def _pack_inputs(x, ln_scale, ln_bias, w_qkv, w_out, b_out):
    """Returns (in_maps for 8 cores, per-batch host bias [1024])."""
    import ml_dtypes

    x = np.ascontiguousarray(np.asarray(x, np.float32))
    ln_scale = np.asarray(ln_scale, np.float32)
    ln_bias = np.asarray(ln_bias, np.float32)
    w_qkv = np.asarray(w_qkv, np.float32)
    w_out = np.asarray(w_out, np.float32)
    b_out = np.asarray(b_out, np.float32)

    ws = w_qkv * ln_scale[:, None]          # fold LN scale into weights
    wq_all = ws[:, 0:1024] * (D ** -0.5)    # fold 1/sqrt(d) into q
    wk_all = ws[:, 1024:2048]
    wv_all = ws[:, 2048:3072]
    qb_all = (ln_bias @ w_qkv[:, 0:1024]) * (D ** -0.5)
    kb_all = ln_bias @ w_qkv[:, 1024:2048]
    vb_all = ln_bias @ w_qkv[:, 2048:3072]
    b_eff = (b_out + vb_all @ w_out).astype(np.float32)  # host-side bias

    in_maps = []
    for core in range(N_CORES):
        b_i, g = core // 2, core % 2
        cs = slice(g * 512, (g + 1) * 512)
        # [dim, 8 heads, 64] -> pairs of heads packed along m
        wq_g = wq_all[:, cs].reshape(DIM, PAIRS, 128)   # [dim, pair, 2*64]
        wk_g = wk_all[:, cs].reshape(DIM, PAIRS, 128)
        # -> [pair, p, kc, m] so that per-pair DMA is contiguous per partition
        wq_p = np.ascontiguousarray(
            wq_g.reshape(KC, 128, PAIRS, 128).transpose(2, 1, 0, 3))
        wk_p = np.ascontiguousarray(
            wk_g.reshape(KC, 128, PAIRS, 128).transpose(1, 0, 2, 3))
        wv_p = np.ascontiguousarray(
            wv_all[:, cs].reshape(KC, 128, 512).transpose(1, 0, 2))
        wo_p = np.ascontiguousarray(
            w_out[cs, :].reshape(PAIRS, 128, DIM).transpose(1, 0, 2))
        qb_p = np.ascontiguousarray(qb_all[cs].reshape(PAIRS, 128))
        kb_p = np.ascontiguousarray(kb_all[cs].reshape(PAIRS, 128))
        xt = np.ascontiguousarray(x[b_i].T)
        in_maps.append({
            "xt": _round_f32r(xt), "wq": _round_f32r(wq_p),
            "wk": _round_f32r(wk_p), "wv": _round_f32r(wv_p),
            "wo": wo_p.astype(ml_dtypes.bfloat16), "qb": qb_p, "kb": kb_p,
        })
    return in_maps, b_eff


def kernel(x, ln_scale, ln_bias, w_qkv, w_out, b_out):
    from concourse.bass_utils import run_bass_kernel_spmd

    nc = get_program()
    in_maps, b_eff = _pack_inputs(x, ln_scale, ln_bias, w_qkv, w_out, b_out)
    trace = bool(os.environ.get("ATTN_KERNEL_TRACE"))
    res = run_bass_kernel_spmd(nc, in_maps, core_ids=list(range(N_CORES)),
                               trace=trace)
    _prog_cache["last_exec_time_ns"] = res.exec_time_ns
    outs = res.results
    out = np.empty((B, N, DIM), np.float32)
    for b in range(B):
        out[b] = outs[2 * b]["out"] + outs[2 * b + 1]["out"] + b_eff
    return out
